# revision 1
# baseline (speedup 1.0000x reference)
"""Distance-correlation loss kernel for trn2 (8 NeuronCores, SPMD).

Math (reference): for F in {X, Y}: a = sqrt(relu(sq_i + sq_j - 2 F F^T) + eps),
row_j = colsum_j / (n-2), tot = sum / ((n-1)(n-2)), A = a - 2*row_j + tot with
zero diagonal; loss = -g_xy / sqrt(g_xx * g_yy + eps), g_PQ = sum(P*Q)/(n(n-3)).

Device strategy per core c (rows 512c..512c+512 of the distance matrix, but
computed TRANSPOSED: tiles aT[j_chunk=128, i=512]):
  pass 1: for each 128-wide j-chunk: 16 accumulating bf16 matmuls
          (stationary = xT strip [128,128], moving = core's xTc [128,512]),
          then ACT(-2*psum + sq_j bias) -> DVE(+sq_i bcast) -> DVE(relu)
          -> ACT(sqrt + eps, accum_out=per-partition colsum) -> ACT(copy -64 -> fp16 cache)
  AllReduce of [2,128,32] partial colsums; rv_shift = -2/(n-2)*C + tot + 64
  pass 2: At = ACT(cache_f16 + rv_shift bias); 3 fused tensor_tensor_reduce ops
          accumulate per-lane partials of sum(At*Bt), sum(At^2), sum(Bt^2).
Host: f64 combine of per-lane partials + bit-exact diagonal removal.
"""

import sys

for _p in ("/opt/trn_rl_repo",):
    if _p not in sys.path:
        sys.path.insert(0, _p)

import numpy as np
import ml_dtypes

import concourse.bass as bass
from concourse import bacc
import concourse.mybir as mybir
import concourse.tile as tile
from concourse.bass_utils import run_bass_kernel_spmd

N = 4096
D = 2048
NCORES = 8
ROWS = N // NCORES          # 512 distance-matrix rows per core (free dim i)
NJ = N // 128               # 32 j-chunks (partition dim of transposed tiles)
NK = D // 128               # 16 contraction chunks
EPS = 1e-18
F32 = mybir.dt.float32
BF16 = mybir.dt.bfloat16
F16 = mybir.dt.float16
AF = mybir.ActivationFunctionType
ALU = mybir.AluOpType

_CACHE = {}


def _build_nc():
    nc = bacc.Bacc(None, num_devices=NCORES, target_bir_lowering=False)

    # ---- I/O ----
    xT = nc.declare_dram_parameter("xT", [D, N], BF16, isOutput=False)
    yT = nc.declare_dram_parameter("yT", [D, N], BF16, isOutput=False)
    xTc = nc.declare_dram_parameter("xTc", [D, ROWS], BF16, isOutput=False)
    yTc = nc.declare_dram_parameter("yTc", [D, ROWS], BF16, isOutput=False)
    # sq[j] reshaped so element (p, nj) = sq[128*nj + p]  (global, same all cores)
    sqjx = nc.declare_dram_parameter("sqjx", [128, NJ], F32, isOutput=False)
    sqjy = nc.declare_dram_parameter("sqjy", [128, NJ], F32, isOutput=False)
    # per-core sq slice for the free axis (rows of this core)
    sqix = nc.declare_dram_parameter("sqix", [1, ROWS], F32, isOutput=False)
    sqiy = nc.declare_dram_parameter("sqiy", [1, ROWS], F32, isOutput=False)

    axh = nc.declare_dram_parameter("axh", [N, ROWS], F16, isOutput=True)
    ayh = nc.declare_dram_parameter("ayh", [N, ROWS], F16, isOutput=True)
    rvs = nc.declare_dram_parameter("rvs", [2, 128, NJ], F32, isOutput=True)
    pp = nc.declare_dram_parameter("pp", [128, 4], F32, isOutput=True)

    with tile.TileContext(nc) as tc:
        import contextlib

        with contextlib.ExitStack() as ctx:
            singles = ctx.enter_context(tc.tile_pool(name="singles", bufs=1))
            strips = ctx.enter_context(tc.tile_pool(name="strips", bufs=8))
            psum = ctx.enter_context(tc.tile_pool(name="psum", bufs=4, space="PSUM"))
            psum1 = ctx.enter_context(tc.tile_pool(name="psum1", bufs=1, space="PSUM"))
            temps = ctx.enter_context(tc.tile_pool(name="temps", bufs=3))
            dram = ctx.enter_context(tc.tile_pool(name="dram", bufs=1, space="DRAM"))

            # ---- residents ----
            def load_resident(name, src, shape, dtype, src_ap=None):
                t = singles.tile(shape, dtype, name=name)
                nc.sync.dma_start(out=t[:], in_=src if src_ap is None else src_ap)
                return t

            xTc_sb = singles.tile([128, NK, ROWS], BF16, name="xTc_sb")
            nc.gpsimd.dma_start(
                out=xTc_sb[:], in_=xTc[:, :].rearrange("(k p) i -> p k i", p=128)
            )
            yTc_sb = singles.tile([128, NK, ROWS], BF16, name="yTc_sb")
            nc.gpsimd.dma_start(
                out=yTc_sb[:], in_=yTc[:, :].rearrange("(k p) i -> p k i", p=128)
            )
            sqjx_sb = singles.tile([128, NJ], F32, name="sqjx_sb")
            nc.gpsimd.dma_start(out=sqjx_sb[:], in_=sqjx[:, :])
            sqjy_sb = singles.tile([128, NJ], F32, name="sqjy_sb")
            nc.gpsimd.dma_start(out=sqjy_sb[:], in_=sqjy[:, :])

            def bcast_load(name, src):
                t = singles.tile([128, ROWS], F32, name=name)
                src_b = bass.AP(
                    tensor=src[:, :].tensor,
                    offset=src[:, :].offset,
                    ap=[[0, 128], [1, ROWS]],
                )
                nc.gpsimd.dma_start(out=t[:], in_=src_b)
                return t

            sqix_sb = bcast_load("sqix_sb", sqix)
            sqiy_sb = bcast_load("sqiy_sb", sqiy)

            # const tiles built by DVE reads of the DMA'd residents: absorbs the
            # DMA-completion waits into these ops so later TS/AC instructions
            # carry at most one sync wait (hardware wait-slot limit).
            eps_sb = singles.tile([128, 1], F32, name="eps_sb")
            nc.vector.tensor_scalar(
                eps_sb[:], sqjx_sb[:, 0:1], 0.0, EPS, op0=ALU.mult, op1=ALU.add
            )
            c64_sb = singles.tile([128, 1], F32, name="c64_sb")
            nc.vector.tensor_scalar(
                c64_sb[:], sqjy_sb[:, 0:1], 0.0, 64.0, op0=ALU.mult, op1=ALU.add
            )
            ones_sb = singles.tile([128, 1], F32, name="ones_sb")
            nc.vector.tensor_scalar(
                ones_sb[:], sqix_sb[:, 0:1], 0.0, 1.0, op0=ALU.mult, op1=ALU.add
            )
            acc = singles.tile([128, 4], F32, name="acc")
            nc.vector.tensor_scalar(
                acc[:], sqiy_sb[:, 0:4], 0.0, 0.0, op0=ALU.mult, op1=ALU.add
            )

            cache_x = singles.tile([128, NJ * ROWS], F16, name="cache_x")
            cache_y = singles.tile([128, NJ * ROWS], F16, name="cache_y")
            cs_xy = singles.tile([128, 2 * NJ], F32, name="cs_xy")

            # ---- pass 1 ----
            def pass1(mT, mTc_sb, sqj_sb, sqi_sb, cache_sb, cs_sb, out_h, tag):
                mT_r = mT[:, :].rearrange("(k p) n -> p k n", p=128)
                for nj in range(NJ):
                    strip = strips.tile([128, NK, 128], BF16, tag="strip")
                    nc.sync.dma_start(
                        out=strip[:],
                        in_=mT_r[:, :, nj * 128 : (nj + 1) * 128],
                    )
                    ps = psum.tile([128, ROWS], F32, tag="mm")
                    for k in range(NK):
                        nc.tensor.matmul(
                            ps[:],
                            lhsT=strip[:, k, :],
                            rhs=mTc_sb[:, k, :],
                            start=(k == 0),
                            stop=(k == NK - 1),
                        )
                    u = temps.tile([128, ROWS], F32, tag="u")
                    nc.vector.tensor_scalar(
                        u[:], ps[:], -2.0, sqj_sb[:, nj : nj + 1],
                        op0=ALU.mult, op1=ALU.add,
                    )
                    v = temps.tile([128, ROWS], F32, tag="v")
                    nc.vector.tensor_add(v[:], u[:], sqi_sb[:])
                    nc.vector.tensor_scalar_max(v[:], v[:], 0.0)
                    a32 = temps.tile([128, ROWS], F32, tag="a32")
                    nc.scalar.activation(
                        a32[:], v[:], AF.Sqrt,
                        bias=eps_sb[:], scale=1.0,
                        accum_out=cs_sb[:, nj : nj + 1],
                    )
                    csl = cache_sb[:, nj * ROWS : (nj + 1) * ROWS]
                    nc.scalar.activation(csl, a32[:], AF.Copy, bias=-64.0, scale=1.0)
                    nc.scalar.dma_start(
                        out=out_h[nj * 128 : (nj + 1) * 128, :], in_=csl
                    )

            import os as _os
            STAGE = int(_os.environ.get("DCOR_STAGE", "4"))
            nc.tensor.ldweights(xTc_sb[:, 0, 0:128])
            pass1(xT, xTc_sb, sqjx_sb, sqix_sb, cache_x, cs_xy[:, 0:NJ], axh, "x")
            if STAGE >= 2:
                nc.tensor.ldweights(yTc_sb[:, 0, 0:128])
                pass1(yT, yTc_sb, sqjy_sb, sqiy_sb, cache_y, cs_xy[:, NJ : 2 * NJ], ayh, "y")

            if STAGE >= 3:
                # ---- AllReduce colsum partials ----
                cc_in = dram.tile([128, 2 * NJ], F32, name="cc_in")
                cc_out = dram.tile([128, 2 * NJ], F32, name="cc_out", addr_space="Shared")
                nc.scalar.dma_start(out=cc_in[:], in_=cs_xy[:])
                import os as _os
                if _os.environ.get("DCOR_NO_CC"):
                    nc.sync.dma_start(out=cc_out[:], in_=cc_in[:])
                else:
                    nc.gpsimd.collective_compute(
                        "AllReduce",
                        ALU.add,
                        replica_groups=[list(range(NCORES))],
                        ins=[cc_in[:]],
                        outs=[cc_out[:]],
                    )
                csf = singles.tile([128, 2 * NJ], F32, name="csf")
                nc.sync.dma_start(out=csf[:], in_=cc_out[:])

                # ---- rv_shift = -2/(n-2)*C + (S/((n-1)(n-2)) + 64) ----
                ones_row = singles.tile([1, 128], F32, name="ones_row")
                nc.vector.tensor_scalar(
                    ones_row[:], sqix_sb[0:1, 0:128], 0.0, 1.0, op0=ALU.mult, op1=ALU.add
                )
                rv_x = singles.tile([128, NJ], F32, name="rv_x")
                rv_y = singles.tile([128, NJ], F32, name="rv_y")
                for m, rv_sb in ((0, rv_x), (1, rv_y)):
                    red = temps.tile([128, 1], F32, tag="red")
                    nc.vector.tensor_reduce(
                        red[:], csf[:, m * NJ : (m + 1) * NJ], mybir.AxisListType.X, ALU.add
                    )
                    ps1 = psum1.tile([1, 1], F32, tag="ps1")
                    nc.tensor.matmul(ps1[:], lhsT=red[:], rhs=ones_sb[:], start=True, stop=True)
                    ts1 = temps.tile([1, 1], F32, tag="ts1")
                    nc.scalar.activation(
                        ts1[:], ps1[:], AF.Identity,
                        bias=c64_sb[0:1, :], scale=1.0 / ((N - 1.0) * (N - 2.0)),
                    )
                    psB = psum1.tile([128, 1], F32, tag="psB")
                    nc.tensor.matmul(psB[:], lhsT=ones_row[:], rhs=ts1[:], start=True, stop=True)
                    nc.vector.tensor_scalar(
                        rv_sb[:], csf[:, m * NJ : (m + 1) * NJ], -2.0 / (N - 2.0), psB[:],
                        op0=ALU.mult, op1=ALU.add,
                    )
                    nc.sync.dma_start(out=rvs[m], in_=rv_sb[:])

            if STAGE >= 4:
                # ---- pass 2 ----
                accs = singles.tile([128, 3 * NJ], F32, name="accs")
                for nj in range(NJ):
                    At = temps.tile([128, ROWS], F32, tag="At")
                    nc.scalar.activation(
                        At[:], cache_x[:, nj * ROWS : (nj + 1) * ROWS], AF.Identity,
                        bias=rv_x[:, nj : nj + 1], scale=1.0,
                    )
                    Bt = temps.tile([128, ROWS], F32, tag="Bt")
                    nc.scalar.activation(
                        Bt[:], cache_y[:, nj * ROWS : (nj + 1) * ROWS], AF.Identity,
                        bias=rv_y[:, nj : nj + 1], scale=1.0,
                    )
                    scrap = temps.tile([128, ROWS], F32, tag="scrap")
                    nc.vector.tensor_mul(scrap[:], At[:], Bt[:])
                    nc.vector.tensor_reduce(
                        accs[:, 0 * NJ + nj : 0 * NJ + nj + 1],
                        scrap[:], mybir.AxisListType.X, ALU.add,
                    )
                    sq_a = temps.tile([128, ROWS], F32, tag="sq_a")
                    nc.scalar.activation(
                        sq_a[:], At[:], AF.Square,
                        accum_out=accs[:, 1 * NJ + nj : 1 * NJ + nj + 1],
                    )
                    sq_b = temps.tile([128, ROWS], F32, tag="sq_b")
                    nc.scalar.activation(
                        sq_b[:], Bt[:], AF.Square,
                        accum_out=accs[:, 2 * NJ + nj : 2 * NJ + nj + 1],
                    )
                for col in range(3):
                    nc.vector.tensor_reduce(
                        acc[:, col : col + 1],
                        accs[:, col * NJ : (col + 1) * NJ],
                        mybir.AxisListType.X,
                        ALU.add,
                    )
                nc.sync.dma_start(out=pp[:, :], in_=acc[:])

    nc.compile()
    return nc


def _get_nc():
    if "nc" not in _CACHE:
        _CACHE["nc"] = _build_nc()
    return _CACHE["nc"]


def kernel(featuresX: np.ndarray, featuresY: np.ndarray) -> np.ndarray:
    X = np.asarray(featuresX, dtype=np.float32).reshape(N, D)
    Y = np.asarray(featuresY, dtype=np.float32).reshape(N, D)

    nc = _get_nc()

    sqx = np.einsum("ij,ij->i", X, X, dtype=np.float32).astype(np.float32)
    sqy = np.einsum("ij,ij->i", Y, Y, dtype=np.float32).astype(np.float32)
    xT = np.ascontiguousarray(X.T).astype(ml_dtypes.bfloat16)
    yT = np.ascontiguousarray(Y.T).astype(ml_dtypes.bfloat16)
    sqjx = np.ascontiguousarray(sqx.reshape(NJ, 128).T)
    sqjy = np.ascontiguousarray(sqy.reshape(NJ, 128).T)

    in_maps = []
    for c in range(NCORES):
        sl = slice(c * ROWS, (c + 1) * ROWS)
        in_maps.append(
            {
                "xT": xT,
                "yT": yT,
                "xTc": np.ascontiguousarray(xT[:, sl]),
                "yTc": np.ascontiguousarray(yT[:, sl]),
                "sqjx": sqjx,
                "sqjy": sqjy,
                "sqix": sqx[sl].reshape(1, ROWS),
                "sqiy": sqy[sl].reshape(1, ROWS),
            }
        )

    _CACHE["in_maps"] = in_maps
    res = run_bass_kernel_spmd(nc, in_maps, list(range(NCORES))).results

    # ---- host combine in f64 ----
    P = np.zeros(3, dtype=np.float64)
    for c in range(NCORES):
        P += res[c]["pp"][:, :3].astype(np.float64).sum(axis=0)

    rv = res[0]["rvs"]  # [2,128,NJ]; rv_flat[128*nj+p] = rv[m,p,nj]
    rvx = np.ascontiguousarray(rv[0].T).reshape(-1)
    rvy = np.ascontiguousarray(rv[1].T).reshape(-1)

    dAB = dAA = dBB = 0.0
    for c in range(NCORES):
        sl = slice(c * ROWS, (c + 1) * ROWS)
        dx16 = res[c]["axh"][sl, :].diagonal().astype(np.float32)
        dy16 = res[c]["ayh"][sl, :].diagonal().astype(np.float32)
        Adiag = (dx16 + rvx[sl]).astype(np.float32).astype(np.float64)
        Bdiag = (dy16 + rvy[sl]).astype(np.float32).astype(np.float64)
        dAB += np.sum(Adiag * Bdiag)
        dAA += np.sum(Adiag * Adiag)
        dBB += np.sum(Bdiag * Bdiag)

    denom = float(N) * (N - 3.0)
    gxy = (P[0] - dAB) / denom
    gxx = (P[1] - dAA) / denom
    gyy = (P[2] - dBB) / denom
    loss = -gxy / np.sqrt(gxx * gyy + EPS)
    return np.array(loss, dtype=np.float32)

